# Initial kernel scaffold
#
"""Distributed causal-attention kernel for one TRN2 chip (8 NeuronCores).

Sharding (hardcoded): core i handles batch i//4 and head-group i%4
(2 heads of 8, head_dim 128).  Each core:
  RMSNorm(x_b) -> QKV proj (its heads) -> causal attention -> O^T
  -> AllToAll within the 4-core batch group (exchange head-shards for
     seq-shards) -> output projection for its 512-row seq slice.
Host gathers the 8 disjoint (512, 1024) slices into (2, 2048, 1024).
"""

import numpy as np

import concourse.bass as bass
import concourse.mybir as mybir
import concourse.tile as tile
from concourse import bacc
from concourse.bass_utils import run_bass_kernel_spmd
from concourse.masks import make_identity, make_causal_mask

F32 = mybir.dt.float32
BF = mybir.dt.bfloat16
AX = mybir.AxisListType.X
AF = mybir.ActivationFunctionType

S = 2048          # sequence length
D = 1024          # model dim
DH = 128          # head dim
HC = 2            # heads per core
FQKV = 3 * HC * DH  # 768 qkv rows per core
P = 128
SB = S // P       # 16 seq blocks
KD = D // P       # 8 d blocks
SA = float(DH) ** -0.5
NEG = -30000.0    # causal mask bias (exp underflows to exactly 0)


def _body(tc):
    nc = tc.nc
    x_ext = nc.declare_dram_parameter("x", [S, D], F32, isOutput=False)
    g_ext = nc.declare_dram_parameter("gamma", [D], F32, isOutput=False)
    wqkv_ext = nc.declare_dram_parameter("w_qkv", [FQKV, D], F32, isOutput=False)
    wout_ext = nc.declare_dram_parameter("w_out", [D, D], F32, isOutput=False)
    out_ext = nc.declare_dram_parameter("out", [S // 4, D], F32, isOutput=True)

    const = tc.tile_pool(name="const", bufs=1)
    dram = tc.tile_pool(name="dram", bufs=1, space="DRAM")
    wpool = tc.tile_pool(name="weights", bufs=1)
    big = tc.tile_pool(name="big", bufs=1)
    xload = tc.tile_pool(name="xload", bufs=3)
    cast = tc.tile_pool(name="cast", bufs=3)
    stat = tc.tile_pool(name="stat", bufs=8)
    spool = tc.tile_pool(name="spool", bufs=2)
    ppool = tc.tile_pool(name="ppool", bufs=5)
    ptp = tc.tile_pool(name="ptp", bufs=3)
    ypool = tc.tile_pool(name="ypool", bufs=2)
    ps_tr = tc.tile_pool(name="ps_tr", bufs=2, space="PSUM")
    ps_mm = tc.tile_pool(name="ps_mm", bufs=2, space="PSUM")
    ps_s = tc.tile_pool(name="ps_s", bufs=2, space="PSUM")
    ps_o = tc.tile_pool(name="ps_o", bufs=2, space="PSUM")

    with const, dram, wpool, big, xload, cast, stat, spool, ppool, ptp, \
            ypool, ps_tr, ps_mm, ps_s, ps_o:
        # ---- constants ----
        ident = const.tile([P, P], BF)
        make_identity(nc, ident)
        masks = []
        for r in range(4):
            mk = const.tile([P, 512], F32, tag=f"mask{r}")
            nc.gpsimd.memset(mk[:, : r * P], 0.0)
            make_causal_mask(nc, mk[:, r * P:(r + 1) * P], mask_val=NEG)
            if r < 3:
                nc.gpsimd.memset(mk[:, (r + 1) * P:], NEG)
            masks.append(mk)
        gamma_kp = const.tile([P, KD], F32)
        nc.sync.dma_start(gamma_kp, g_ext.ap().rearrange("(o p) -> p o", p=P))

        # ---- weight prep: transpose to [d_inner, d_outer, f] bf16 ----
        wqkvT = wpool.tile([P, KD, FQKV], BF)
        for fb in range(FQKV // P):
            wn = xload.tile([P, D], F32, tag="wload")
            nc.sync.dma_start(wn, wqkv_ext[fb * P:(fb + 1) * P, :])
            wnb = cast.tile([P, D], BF, tag="wcast")
            nc.vector.tensor_copy(wnb, wn)
            for k in range(KD):
                pst = ps_tr.tile([P, 512], F32)
                nc.tensor.transpose(pst[:, :P], wnb[:, k * P:(k + 1) * P], ident)
                nc.vector.tensor_scalar_mul(
                    wqkvT[:, k, fb * P:(fb + 1) * P], pst[:, :P],
                    gamma_kp[:, k:k + 1])
        woT = wpool.tile([P, KD, D], BF)
        for cb in range(D // P):
            wn = xload.tile([P, D], F32, tag="wload")
            nc.sync.dma_start(wn, wout_ext[cb * P:(cb + 1) * P, :])
            wnb = cast.tile([P, D], BF, tag="wcast")
            nc.vector.tensor_copy(wnb, wn)
            for k in range(KD):
                pst = ps_tr.tile([P, 512], F32)
                nc.tensor.transpose(pst[:, :P], wnb[:, k * P:(k + 1) * P], ident)
                nc.vector.tensor_copy(woT[:, k, cb * P:(cb + 1) * P], pst[:, :P])

        # ---- norm: xn = x * (32 / ||x||) ; write bf16 to DRAM ----
        xn_dram = dram.tile([S, D], BF)
        for si in range(SB):
            xt = xload.tile([P, D], F32, tag="xt")
            nc.sync.dma_start(xt, x_ext[si * P:(si + 1) * P, :])
            sq = cast.tile([P, D], BF, tag="sq")
            ss = stat.tile([P, 1], F32, tag="ss")
            nc.scalar.activation(sq, xt, AF.Square, accum_out=ss)
            sc = stat.tile([P, 1], F32, tag="sc")
            nc.scalar.activation(sc, ss, AF.Rsqrt, scale=1.0 / D)
            xnb = cast.tile([P, D], BF, tag="xnb")
            nc.vector.tensor_scalar_mul(xnb, xt, sc)
            nc.sync.dma_start(xn_dram[si * P:(si + 1) * P, :], xnb)

        # ---- transpose back: xnT [d_inner, d_outer, s] ----
        xnT = big.tile([P, KD, S], BF)
        for k in range(KD):
            for c in range(4):
                nc.sync.dma_start_transpose(
                    xnT[:, k, c * 512:(c + 1) * 512],
                    xn_dram[c * 512:(c + 1) * 512, k * P:(k + 1) * P])

        # ---- QKV projection: qkvT [dh, {q0,q1,k0,k1,v0,v1}, s] ----
        qkvT = wpool.tile([P, 6, S], BF)
        for fb in range(6):
            for c in range(4):
                pm = ps_mm.tile([P, 512], F32)
                for k in range(KD):
                    nc.tensor.matmul(
                        pm, wqkvT[:, k, fb * P:(fb + 1) * P],
                        xnT[:, k, c * 512:(c + 1) * 512],
                        start=(k == 0), stop=(k == KD - 1))
                nc.vector.tensor_copy(qkvT[:, fb, c * 512:(c + 1) * 512], pm)

        # ---- V to natural layout [s_inner, s_outer, (h, dh)] ----
        v_sb = wpool.tile([P, SB, HC * DH], BF)
        for h in range(HC):
            for sb in range(SB):
                pst = ps_tr.tile([P, 512], F32)
                nc.tensor.transpose(
                    pst[:, :P], qkvT[:, 4 + h, sb * P:(sb + 1) * P], ident)
                nc.vector.tensor_copy(
                    v_sb[:, sb, h * DH:(h + 1) * DH], pst[:, :P])

        # ---- causal attention (flash-free: full rows fit) ----
        oT = wpool.tile([P, HC, S], BF)
        for h in range(HC):
            for a in range(4):            # 512-row super-blocks
                ptiles = []
                L = (a + 1) * 512
                for r in range(4):
                    qi = 4 * a + r
                    ssb = spool.tile([P, S], F32, tag="ssb")
                    for jc in range(a + 1):
                        ps = ps_s.tile([P, 512], F32)
                        nc.tensor.matmul(
                            ps, qkvT[:, h, qi * P:(qi + 1) * P],
                            qkvT[:, 2 + h, jc * 512:(jc + 1) * 512],
                            start=True, stop=True)
                        if jc == a:
                            nc.vector.tensor_add(
                                ssb[:, jc * 512:(jc + 1) * 512], ps, masks[r])
                        else:
                            nc.vector.tensor_copy(
                                ssb[:, jc * 512:(jc + 1) * 512], ps)
                    m = stat.tile([P, 1], F32, tag="m")
                    nc.vector.reduce_max(m, ssb[:, :L], axis=AX)
                    negm = stat.tile([P, 1], F32, tag="negm")
                    nc.vector.tensor_scalar_mul(negm, m, -SA)
                    pt = ppool.tile([P, S], BF, tag="p")
                    l = stat.tile([P, 1], F32, tag="l")
                    nc.scalar.activation(pt[:, :L], ssb[:, :L], AF.Exp,
                                         bias=negm, scale=SA, accum_out=l)
                    rl = stat.tile([P, 1], F32, tag="rl")
                    nc.vector.reciprocal(rl, l)
                    nc.vector.tensor_scalar_mul(pt[:, :L], pt[:, :L], rl)
                    ptiles.append(pt)
                po = ps_o.tile([P, 512], F32)
                nj = 4 * (a + 1)
                for jb in range(nj):
                    pst = ps_tr.tile([P, 512], F32)
                    for r in range(4):
                        nc.tensor.transpose(
                            pst[:, r * P:(r + 1) * P],
                            ptiles[r][:, jb * P:(jb + 1) * P], ident)
                    ptt = ptp.tile([P, 512], BF)
                    nc.vector.tensor_copy(ptt, pst)
                    nc.tensor.matmul(
                        po, v_sb[:, jb, h * DH:(h + 1) * DH], ptt,
                        start=(jb == 0), stop=(jb == nj - 1))
                nc.vector.tensor_copy(oT[:, h, a * 512:(a + 1) * 512], po)

        # ---- AllToAll within 4-core batch group ----
        a2a_in = dram.tile([4 * HC * DH, 512], BF)
        a2a_out = dram.tile([4 * HC * DH, 512], BF)
        for d in range(4):
            for h in range(HC):
                nc.sync.dma_start(
                    a2a_in[d * HC * DH + h * DH: d * HC * DH + (h + 1) * DH, :],
                    oT[:, h, d * 512:(d + 1) * 512])
        nc.gpsimd.collective_compute(
            "AllToAll", mybir.AluOpType.bypass,
            replica_groups=[[0, 1, 2, 3], [4, 5, 6, 7]],
            ins=[a2a_in[:, :].opt()], outs=[a2a_out[:, :].opt()])
        ofT = big.tile([P, KD, 512], BF)
        nc.sync.dma_start(ofT, a2a_out[:, :].rearrange("(o p) s -> p o s", p=P))

        # ---- output projection for my 512-row slice ----
        for sb in range(4):
            y = ypool.tile([P, D], F32)
            for cc in range(2):
                pm = ps_mm.tile([P, 512], F32)
                for k in range(KD):
                    nc.tensor.matmul(
                        pm, ofT[:, k, sb * P:(sb + 1) * P],
                        woT[:, k, cc * 512:(cc + 1) * 512],
                        start=(k == 0), stop=(k == KD - 1))
                nc.vector.tensor_copy(y[:, cc * 512:(cc + 1) * 512], pm)
            nc.sync.dma_start(out_ext[sb * P:(sb + 1) * P, :], y)


def build():
    nc = bacc.Bacc(None, target_bir_lowering=False)
    with tile.TileContext(nc) as tc:
        _body(tc)
    return nc


_NC = None


def run(inputs, trace=False):
    global _NC
    x = np.ascontiguousarray(np.asarray(inputs["x"], np.float32))
    gamma = np.ascontiguousarray(np.asarray(inputs["gamma"], np.float32))
    w_qkv = np.ascontiguousarray(np.asarray(inputs["w_qkv"], np.float32))
    w_out = np.ascontiguousarray(np.asarray(inputs["w_out"], np.float32))
    if _NC is None:
        _NC = build()
    in_maps = []
    for i in range(8):
        b, g = i // 4, i % 4
        rows = np.ascontiguousarray(np.concatenate([
            w_qkv[256 * g:256 * (g + 1)],
            w_qkv[1024 + 256 * g:1024 + 256 * (g + 1)],
            w_qkv[2048 + 256 * g:2048 + 256 * (g + 1)]], axis=0))
        in_maps.append({"x": np.ascontiguousarray(x[b]), "gamma": gamma,
                        "w_qkv": rows, "w_out": w_out})
    br = run_bass_kernel_spmd(_NC, in_maps, list(range(8)), trace=trace)
    out = np.empty((2, S, D), np.float32)
    for i in range(8):
        b, r = i // 4, i % 4
        out[b, r * 512:(r + 1) * 512, :] = br.results[i]["out"]
    return out, br


def kernel(**inputs):
    out, _ = run(inputs, trace=False)
    return out


# revision 14
# speedup vs baseline: 1.0009x; 1.0009x over previous
"""Distributed causal-attention kernel for one TRN2 chip (8 NeuronCores).

Sharding (hardcoded): core i handles batch i//4 and head-group i%4
(2 heads of 8, head_dim 128).  Each core:
  RMSNorm(x_b) -> QKV proj (its heads) -> causal attention -> O^T
  -> AllToAll within the 4-core batch group (exchange head-shards for
     seq-shards) -> output projection for its 512-row seq slice.
Host gathers the 8 disjoint (512, 1024) slices into (2, 2048, 1024).
"""

import numpy as np

import concourse.bass as bass
import concourse.mybir as mybir
import concourse.tile as tile
from concourse import bacc
from concourse.bass_utils import run_bass_kernel_spmd
from concourse.masks import make_identity, make_causal_mask

F32 = mybir.dt.float32
BF = mybir.dt.bfloat16
AX = mybir.AxisListType.X
AF = mybir.ActivationFunctionType

S = 2048          # sequence length
D = 1024          # model dim
DH = 128          # head dim
HC = 2            # heads per core
FQKV = 3 * HC * DH  # 768 qkv rows per core
P = 128
SB = S // P       # 16 seq blocks
KD = D // P       # 8 d blocks
SA = float(DH) ** -0.5
NEG = -30000.0    # causal mask bias (exp underflows to exactly 0)


def _body(tc):
    nc = tc.nc
    x_ext = nc.declare_dram_parameter("x", [S, D], F32, isOutput=False)
    g_ext = nc.declare_dram_parameter("gamma", [D], F32, isOutput=False)
    wqkv_ext = nc.declare_dram_parameter("w_qkv", [FQKV, D], F32, isOutput=False)
    wout_ext = nc.declare_dram_parameter("w_out", [D, D], F32, isOutput=False)
    out_ext = nc.declare_dram_parameter("out", [S // 4, D], F32, isOutput=True)

    from contextlib import ExitStack
    with ExitStack() as ctx:
        const = ctx.enter_context(tc.tile_pool(name="const", bufs=1))
        dram = ctx.enter_context(tc.tile_pool(name="dram", bufs=1, space="DRAM"))
        wpool = ctx.enter_context(tc.tile_pool(name="weights", bufs=1))
        big = ctx.enter_context(tc.tile_pool(name="big", bufs=1))
        xload = ctx.enter_context(tc.tile_pool(name="xload", bufs=3))
        cast = ctx.enter_context(tc.tile_pool(name="cast", bufs=3))
        stat = ctx.enter_context(tc.tile_pool(name="stat", bufs=8))
        spool = ctx.enter_context(tc.tile_pool(name="spool", bufs=2))
        ppool = ctx.enter_context(tc.tile_pool(name="ppool", bufs=5))
        ptp = ctx.enter_context(tc.tile_pool(name="ptp", bufs=3))
        ypool = ctx.enter_context(tc.tile_pool(name="ypool", bufs=2))
        ps_tr = ctx.enter_context(tc.tile_pool(name="ps_tr", bufs=2, space="PSUM"))
        ps_mm = ctx.enter_context(tc.tile_pool(name="ps_mm", bufs=2, space="PSUM"))
        ps_s = ctx.enter_context(tc.tile_pool(name="ps_s", bufs=2, space="PSUM"))
        ps_o = ctx.enter_context(tc.tile_pool(name="ps_o", bufs=2, space="PSUM"))
        # ---- constants ----
        ident = const.tile([P, P], BF)
        make_identity(nc, ident)
        masks = []
        for r in range(4):
            mk = const.tile([P, 512], F32, tag=f"mask{r}")
            if r > 0:
                nc.gpsimd.memset(mk[:, : r * P], 0.0)
            make_causal_mask(nc, mk[:, r * P:(r + 1) * P], mask_val=NEG)
            if r < 3:
                nc.gpsimd.memset(mk[:, (r + 1) * P:], NEG)
            masks.append(mk)
        gamma_kp = const.tile([P, KD], F32)
        nc.sync.dma_start(gamma_kp, g_ext.ap().rearrange("(o p) -> p o", p=P))

        # ---- weight prep: transpose to [d_inner, d_outer, f] bf16 ----
        wqkvT = wpool.tile([P, KD, FQKV], BF)
        for fb in range(FQKV // P):
            wn = xload.tile([P, D], F32, tag="wload")
            nc.sync.dma_start(wn, wqkv_ext[fb * P:(fb + 1) * P, :])
            wnb = cast.tile([P, D], BF, tag="wcast")
            nc.vector.tensor_copy(wnb, wn)
            for k in range(KD):
                pst = ps_tr.tile([P, 512], BF)
                nc.tensor.transpose(pst[:, :P], wnb[:, k * P:(k + 1) * P], ident)
                nc.vector.tensor_scalar_mul(
                    wqkvT[:, k, fb * P:(fb + 1) * P], pst[:, :P],
                    gamma_kp[:, k:k + 1])
        woT = wpool.tile([P, KD, D], BF)
        for cb in range(D // P):
            wn = xload.tile([P, D], F32, tag="wload")
            nc.sync.dma_start(wn, wout_ext[cb * P:(cb + 1) * P, :])
            wnb = cast.tile([P, D], BF, tag="wcast")
            nc.vector.tensor_copy(wnb, wn)
            for k in range(KD):
                pst = ps_tr.tile([P, 512], BF)
                nc.tensor.transpose(pst[:, :P], wnb[:, k * P:(k + 1) * P], ident)
                nc.vector.tensor_copy(woT[:, k, cb * P:(cb + 1) * P], pst[:, :P])

        # ---- norm: xn = x * (32 / ||x||) ; write bf16 to DRAM ----
        xn_dram = dram.tile([S, D], BF)
        for si in range(SB):
            xt = xload.tile([P, D], F32, tag="xt")
            nc.sync.dma_start(xt, x_ext[si * P:(si + 1) * P, :])
            sq = cast.tile([P, D], BF, tag="sq")
            ss = stat.tile([P, 1], F32, tag="ss")
            nc.scalar.activation(sq, xt, AF.Square, accum_out=ss)
            sl = stat.tile([P, 1], F32, tag="sl")
            nc.scalar.activation(sl, ss, AF.Sqrt, scale=1.0 / D)
            sc = stat.tile([P, 1], F32, tag="sc")
            nc.vector.reciprocal(sc, sl)
            xnb = cast.tile([P, D], BF, tag="xnb")
            nc.vector.tensor_scalar_mul(xnb, xt, sc)
            nc.sync.dma_start(xn_dram[si * P:(si + 1) * P, :], xnb)

        # ---- transpose back: xnT [d_inner, d_outer, s] ----
        xnT = big.tile([P, KD, S], BF)
        for k in range(KD):
            for c in range(4):
                nc.sync.dma_start_transpose(
                    xnT[:, k, c * 512:(c + 1) * 512],
                    xn_dram[c * 512:(c + 1) * 512, k * P:(k + 1) * P])

        # ---- QKV projection: qkvT [dh, {q0,q1,k0,k1,v0,v1}, s] ----
        qkvT = wpool.tile([P, 6, S], BF)
        for fb in range(6):
            for c in range(4):
                pm = ps_mm.tile([P, 512], F32)
                for k in range(KD):
                    nc.tensor.matmul(
                        pm, wqkvT[:, k, fb * P:(fb + 1) * P],
                        xnT[:, k, c * 512:(c + 1) * 512],
                        start=(k == 0), stop=(k == KD - 1))
                nc.vector.tensor_copy(qkvT[:, fb, c * 512:(c + 1) * 512], pm)

        # ---- V to natural layout [s_inner, s_outer, (h, dh)] ----
        v_sb = wpool.tile([P, SB, HC * DH], BF)
        for h in range(HC):
            for sb in range(SB):
                pst = ps_tr.tile([P, 512], BF)
                nc.tensor.transpose(
                    pst[:, :P], qkvT[:, 4 + h, sb * P:(sb + 1) * P], ident)
                nc.vector.tensor_copy(
                    v_sb[:, sb, h * DH:(h + 1) * DH], pst[:, :P])

        import os
        if os.environ.get("KSTAGE") == "qkv":
            for sb in range(4):
                y = ypool.tile([P, D], F32)
                nc.vector.tensor_copy(y, qkvT[:, sb, :D])
                nc.sync.dma_start(out_ext[sb * P:(sb + 1) * P, :], y)
            return

        # ---- causal attention (flash-free: full rows fit) ----
        a2a_in = dram.tile([8 * HC * DH, 256], F32)
        a2a_out = dram.tile([8 * HC * DH, 256], F32)
        for h in range(HC):
            for a in range(4):            # 512-row super-blocks
                ptiles = []
                L = (a + 1) * 512
                for r in range(4):
                    qi = 4 * a + r
                    ssb = spool.tile([P, S], F32, tag="ssb")
                    for jc in range(a + 1):
                        ps = ps_s.tile([P, 512], F32)
                        nc.tensor.matmul(
                            ps, qkvT[:, h, qi * P:(qi + 1) * P],
                            qkvT[:, 2 + h, jc * 512:(jc + 1) * 512],
                            start=True, stop=True)
                        if jc == a:
                            nc.vector.tensor_add(
                                ssb[:, jc * 512:(jc + 1) * 512], ps, masks[r])
                        else:
                            nc.vector.tensor_copy(
                                ssb[:, jc * 512:(jc + 1) * 512], ps)
                    m = stat.tile([P, 1], F32, tag="m")
                    nc.vector.reduce_max(m, ssb[:, :L], axis=AX)
                    negm = stat.tile([P, 1], F32, tag="negm")
                    nc.vector.tensor_scalar_mul(negm, m, -SA)
                    pt = ppool.tile([P, S], BF, tag="p")
                    l = stat.tile([P, 1], F32, tag="l")
                    nc.scalar.activation(pt[:, :L], ssb[:, :L], AF.Exp,
                                         bias=negm, scale=SA, accum_out=l)
                    rl = stat.tile([P, 1], F32, tag="rl")
                    nc.vector.reciprocal(rl, l)
                    nc.vector.tensor_scalar_mul(pt[:, :L], pt[:, :L], rl)
                    ptiles.append(pt)
                po = ps_o.tile([P, 512], F32)
                nj = 4 * (a + 1)
                for jb in range(nj):
                    pst = ps_tr.tile([P, 512], BF)
                    for r in range(4):
                        nc.tensor.transpose(
                            pst[:, r * P:(r + 1) * P],
                            ptiles[r][:, jb * P:(jb + 1) * P], ident)
                    ptt = ptp.tile([P, 512], BF)
                    nc.vector.tensor_copy(ptt, pst)
                    nc.tensor.matmul(
                        po, v_sb[:, jb, h * DH:(h + 1) * DH], ptt,
                        start=(jb == 0), stop=(jb == nj - 1))
                o512 = ypool.tile([P, 512], F32, tag="o512")
                nc.vector.tensor_copy(o512, po)
                for dd in range(2):
                    d = 2 * a + dd
                    nc.sync.dma_start(
                        a2a_in[d * HC * DH + h * DH: d * HC * DH + (h + 1) * DH, :],
                        o512[:, dd * 256:(dd + 1) * 256])

        # ---- 8-core AllToAll: chunk d = my O^T rows [256d, 256d+256) ----
        # Receiver i gets, for each sender c=(batch c//4, heads c%4), that
        # sender's O^T slice for rows [256i, 256i+256) of sender's batch.
        # So a2a_out[0:1024] = full-head O^T of batch 0, rows slice i;
        # a2a_out[1024:2048] = same for batch 1.
        if os.environ.get("KSTAGE") != "noa2a":
            nc.gpsimd.collective_compute(
                "AllToAll", mybir.AluOpType.bypass,
                replica_groups=[[0, 1, 2, 3, 4, 5, 6, 7]],
                ins=[a2a_in[:, :].opt()], outs=[a2a_out[:, :].opt()])
        else:
            nc.sync.dma_start(a2a_out[:, :], a2a_in[:, :])

        # ---- output projection: 256 rows for each batch ----
        for b in range(2):
            ofT32 = big.tile([P, KD, 256], F32, tag="ofT32")
            nc.sync.dma_start(
                ofT32, a2a_out[b * D:(b + 1) * D, :].rearrange(
                    "(o p) s -> p o s", p=P))
            ofT = big.tile([P, KD, 256], BF, tag="ofT")
            nc.vector.tensor_copy(ofT, ofT32)
            for sb in range(2):
                y = ypool.tile([P, D], F32)
                for cc in range(2):
                    pm = ps_mm.tile([P, 512], F32)
                    for k in range(KD):
                        nc.tensor.matmul(
                            pm, ofT[:, k, sb * P:(sb + 1) * P],
                            woT[:, k, cc * 512:(cc + 1) * 512],
                            start=(k == 0), stop=(k == KD - 1))
                    nc.vector.tensor_copy(y[:, cc * 512:(cc + 1) * 512], pm)
                nc.sync.dma_start(
                    out_ext[b * 256 + sb * P: b * 256 + (sb + 1) * P, :], y)


def build():
    nc = bacc.Bacc(None, target_bir_lowering=False)
    with tile.TileContext(nc) as tc:
        _body(tc)
    nc.compile()
    return nc


_NC = None


def make_in_maps(inputs):
    x = np.ascontiguousarray(np.asarray(inputs["x"], np.float32))
    gamma = np.ascontiguousarray(np.asarray(inputs["gamma"], np.float32))
    w_qkv = np.ascontiguousarray(np.asarray(inputs["w_qkv"], np.float32))
    w_out = np.ascontiguousarray(np.asarray(inputs["w_out"], np.float32))
    in_maps = []
    for i in range(8):
        b, g = i // 4, i % 4
        rows = np.ascontiguousarray(np.concatenate([
            w_qkv[256 * g:256 * (g + 1)],
            w_qkv[1024 + 256 * g:1024 + 256 * (g + 1)],
            w_qkv[2048 + 256 * g:2048 + 256 * (g + 1)]], axis=0))
        in_maps.append({"x": np.ascontiguousarray(x[b]), "gamma": gamma,
                        "w_qkv": rows, "w_out": w_out})
    return in_maps


def run(inputs, trace=False):
    global _NC
    if _NC is None:
        _NC = build()
    in_maps = make_in_maps(inputs)
    br = run_bass_kernel_spmd(_NC, in_maps, list(range(8)), trace=trace)
    out = np.empty((2, S, D), np.float32)
    for i in range(8):
        o = br.results[i]["out"]
        out[0, i * 256:(i + 1) * 256, :] = o[:256]
        out[1, i * 256:(i + 1) * 256, :] = o[256:]
    return out, br


def kernel(**inputs):
    out, _ = run(inputs, trace=False)
    return out
